# revision 3
# baseline (speedup 1.0000x reference)
"""Distributed causal attention head for Trainium2 (8 NeuronCores).

Problem: inputs [8,2048,768] f32, attention_mask [1,2048,2048] int32,
Q/K/V [768,64] f32 -> out [8,2048,64] f32
  q,k,v = x@Q, x@K, x@V ; w = q k^T / 8 masked ; out = softmax(w) @ v

Sharding: data-parallel over batch B=8 -> one batch element per core.
Per core everything is computed transposed (d on partitions) so that the
score matrix lands in [keys, queries] layout, letting the AV matmul use
it directly as the moving operand with v as the stationary operand.  The
softmax denominator comes for free from a ones-column appended to v.
Causal (block-sparse) structure of the mask is exploited by skipping
fully-masked 128x512 score blocks entirely; partially-masked blocks are
multiplied by mask patterns shipped from the host (exp of a valid score
is finite, so masking after exp with 0/1 weights is exact).
"""

import sys

if "/opt/trn_rl_repo" not in sys.path:
    sys.path.insert(0, "/opt/trn_rl_repo")

import numpy as np

import concourse.bacc as bacc
import concourse.mybir as mybir
from concourse import tile
from concourse.bass_utils import run_bass_kernel_spmd

B, S, E, D = 8, 2048, 768, 64
EC = E // 128          # 6 e-chunks
NJ = 4                 # q blocks of 512
QW = S // NJ           # 512
NI = 16                # ks blocks of 128
KW = S // NI           # 128
SCALE = 1.0 / 8.0      # 1/sqrt(64)

F32 = mybir.dt.float32
BF16 = mybir.dt.bfloat16


def _classify_mask(mask):
    """mask: [S,S] int (q,k indexed). Returns (blocks, patterns).

    blocks[J] = list of (i, pat_idx|None) for ks-blocks included for
    q-block J.  patterns: list of [128, QW] f32 arrays (wT layout:
    partition=ks, free=q) for partially masked blocks.
    """
    mb = (mask != 0).reshape(NJ, QW, NI, KW)
    sums = mb.sum(axis=(1, 3))  # [NJ, NI]
    patterns = []
    pat_ids = {}
    blocks = []
    for J in range(NJ):
        row = []
        for i in range(NI):
            s = int(sums[J, i])
            if s == 0:
                continue
            if s == QW * KW:
                row.append((i, None))
                continue
            pat = np.ascontiguousarray(
                mb[J, :, i, :].T.astype(np.float32)
            )  # [KW, QW]
            key = pat.tobytes()
            if key not in pat_ids:
                pat_ids[key] = len(patterns)
                patterns.append(pat)
            row.append((i, pat_ids[key]))
        if not row:
            raise ValueError(f"q-block {J} has no valid keys (all-masked rows)")
        blocks.append(row)
    return blocks, patterns


def _build(blocks, n_pat):
    """Build the per-core Bass graph (identical on all 8 cores)."""
    nc = bacc.Bacc("TRN2", target_bir_lowering=False, debug=False, num_devices=B)

    xT = nc.declare_dram_parameter("xT", [E, S], F32, isOutput=False)
    wqkv = nc.declare_dram_parameter("wqkv", [E, 192], F32, isOutput=False)
    if n_pat:
        masks = nc.declare_dram_parameter("masks", [128, n_pat * QW], F32, isOutput=False)
    out = nc.declare_dram_parameter("out", [D, S], F32, isOutput=True)

    xT_v = xT.ap().rearrange("(a p) s -> p a s", p=128)       # [128, EC, S]
    w_v = wqkv.ap().rearrange("(a p) d -> p a d", p=128)      # [128, EC, 192]

    EXP = mybir.ActivationFunctionType.Exp
    PSUM = "PSUM"

    with tile.TileContext(nc) as tc:
        with tc.tile_pool(name="perm", bufs=1) as perm, \
             tc.tile_pool(name="vpool", bufs=NI) as vpool, \
             tc.tile_pool(name="expp", bufs=3) as expp, \
             tc.tile_pool(name="smallp", bufs=2) as smallp:

            xt_sb = perm.tile([128, EC, S], BF16, tag="xt")
            w_sb = perm.tile([128, EC, 192], BF16, tag="w")
            qk_sb = perm.tile([128, S], BF16, tag="qk")   # rows 0:64 qT, 64:128 kT
            kt_sb = perm.tile([64, S], BF16, tag="kt")
            vt_sb = perm.tile([64, S], BF16, tag="vt")
            ones_sb = perm.tile([1, D], BF16, tag="ones")
            if n_pat:
                mask_sb = perm.tile([128, n_pat, QW], BF16, tag="masks")

            # ---- loads (SWDGE casts f32 -> bf16 in flight) ----
            for c in range(EC):
                nc.gpsimd.dma_start(xt_sb[:, c, :], xT_v[:, c, :])
            nc.gpsimd.dma_start(w_sb[:], w_v[:])
            if n_pat:
                nc.gpsimd.dma_start(
                    mask_sb[:], masks.ap().rearrange("p (m s) -> p m s", s=QW)
                )
            nc.vector.memset(ones_sb[:], 1.0)

            # ---- projections ----
            with tc.tile_pool(name="projp", bufs=1, space=PSUM) as projp:
                qkp = projp.tile([128, S], F32, tag="qkp")   # 4 banks
                vtp = projp.tile([64, S], F32, tag="vtp")    # 4 banks
                for c in range(EC):
                    for h in range(4):
                        sl = slice(h * 512, (h + 1) * 512)
                        nc.tensor.matmul(
                            qkp[:, sl], w_sb[:, c, 0:128], xt_sb[:, c, sl],
                            start=(c == 0), stop=(c == EC - 1),
                        )
                    for h in range(4):
                        sl = slice(h * 512, (h + 1) * 512)
                        nc.tensor.matmul(
                            vtp[:, sl], w_sb[:, c, 128:192], xt_sb[:, c, sl],
                            start=(c == 0), stop=(c == EC - 1),
                        )
                nc.vector.tensor_copy(qk_sb[:], qkp[:])
                nc.scalar.activation(
                    vt_sb[:], vtp[:], mybir.ActivationFunctionType.Copy
                )

            # kT to partitions 0:63 (cross-partition move => DMA)
            nc.sync.dma_start(kt_sb[:], qk_sb[64:128, :])

            # v natural layout via xbar transpose, plus a ones column
            v_tiles = []
            for t in range(NI):
                vt_t = vpool.tile([128, D + 1], BF16, tag="v")
                nc.sync.dma_start(
                    vt_t[:, 0:D], vt_sb[:, t * KW:(t + 1) * KW], transpose=True
                )
                nc.vector.memset(vt_t[:, D:D + 1], 1.0)
                v_tiles.append(vt_t)

            # ---- main attention loop ----
            with tc.tile_pool(name="wp", bufs=2, space=PSUM) as wp, \
                 tc.tile_pool(name="op", bufs=2, space=PSUM) as op, \
                 tc.tile_pool(name="bp", bufs=2, space=PSUM) as bp:
                for J in range(NJ):
                    qsl = slice(J * QW, (J + 1) * QW)
                    row = blocks[J]
                    strips = [row[t:t + 2] for t in range(0, len(row), 2)]
                    o_ps = op.tile([D + 1, QW], F32, tag="o")
                    n_av = 0
                    for strip in strips:
                        w_ps = wp.tile([128, QW * len(strip)], F32, tag="w")
                        et = expp.tile([128, QW * len(strip)], BF16, tag="e")
                        for s_idx, (i, _) in enumerate(strip):
                            nc.tensor.matmul(
                                w_ps[:, s_idx * QW:(s_idx + 1) * QW],
                                kt_sb[:, i * KW:(i + 1) * KW],
                                qk_sb[0:64, qsl],
                                start=True, stop=True,
                            )
                        nc.scalar.activation(et[:], w_ps[:], EXP, scale=SCALE)
                        for s_idx, (i, pat) in enumerate(strip):
                            if pat is not None:
                                esl = slice(s_idx * QW, (s_idx + 1) * QW)
                                nc.vector.tensor_mul(
                                    et[:, esl], et[:, esl], mask_sb[:, pat, :]
                                )
                        for s_idx, (i, _) in enumerate(strip):
                            n_av += 1
                            nc.tensor.matmul(
                                o_ps[:],
                                v_tiles[i][:, 0:D + 1],
                                et[:, s_idx * QW:(s_idx + 1) * QW],
                                start=(n_av == 1), stop=(n_av == len(row)),
                            )
                    # normalize: out[d,q] = o_ps[d,q] * (1/o_ps[64,q])
                    rt = smallp.tile([1, QW], BF16, tag="r")
                    with nc.allow_low_precision("softmax denom recip in bf16"):
                        nc.vector.reciprocal(rt[:], o_ps[D:D + 1, :])
                    b_ps = bp.tile([D, QW], F32, tag="b")
                    nc.tensor.matmul(b_ps[:], ones_sb[:], rt[:], start=True, stop=True)
                    ob = smallp.tile([D, QW], BF16, tag="ob")
                    nc.scalar.activation(
                        ob[:], o_ps[0:D, :], mybir.ActivationFunctionType.Copy
                    )
                    of = smallp.tile([D, QW], F32, tag="of")
                    nc.vector.tensor_mul(of[:], ob[:], b_ps[:])
                    nc.sync.dma_start(out.ap()[:, qsl], of[:])

    nc.compile()
    return nc


_CACHE = {}


def kernel(inputs, attention_mask, Q, K, V):
    inputs = np.asarray(inputs, dtype=np.float32)
    Q = np.asarray(Q, dtype=np.float32)
    K = np.asarray(K, dtype=np.float32)
    V = np.asarray(V, dtype=np.float32)
    mask = np.asarray(attention_mask)
    assert inputs.shape == (B, S, E)
    assert mask.shape[-2:] == (S, S)

    blocks, patterns = _classify_mask(mask.reshape(S, S))
    n_pat = len(patterns)

    key = (tuple(tuple(r) for r in blocks), n_pat)
    if key not in _CACHE:
        _CACHE[key] = _build(blocks, n_pat)
    nc = _CACHE[key]

    wqkv = np.ascontiguousarray(np.concatenate([Q, K, V], axis=1))  # [E, 192]
    if n_pat:
        mask_packed = np.ascontiguousarray(
            np.concatenate(patterns, axis=1)
        )  # [128, n_pat*QW]

    in_maps = []
    for b in range(B):
        m = {
            "xT": np.ascontiguousarray(inputs[b].T),  # [E, S]
            "wqkv": wqkv,
        }
        if n_pat:
            m["masks"] = mask_packed
        in_maps.append(m)

    res = run_bass_kernel_spmd(nc, in_maps, core_ids=list(range(B)))
    global _LAST_RESULTS
    _LAST_RESULTS = res
    out = np.stack([res.results[b]["out"].T for b in range(B)], axis=0)
    return np.ascontiguousarray(out.astype(np.float32))


_LAST_RESULTS = None


if __name__ == "__main__":
    rng = np.random.default_rng(0)
    x = rng.standard_normal((B, S, E), dtype=np.float32)
    am = np.tril(np.ones((S, S), dtype=np.int32))[None]
    Q = rng.standard_normal((E, D), dtype=np.float32) * 0.01
    K = rng.standard_normal((E, D), dtype=np.float32) * 0.01
    V = rng.standard_normal((E, D), dtype=np.float32) * 0.01
    o = kernel(x, am, Q, K, V)
    print(o.shape, o.dtype)


# revision 5
# speedup vs baseline: 1.0566x; 1.0566x over previous
"""Distributed causal attention head for Trainium2 (8 NeuronCores).

Problem: inputs [8,2048,768] f32, attention_mask [1,2048,2048] int32,
Q/K/V [768,64] f32 -> out [8,2048,64] f32
  q,k,v = x@Q, x@K, x@V ; w = q k^T / 8 masked ; out = softmax(w) @ v

Sharding: data-parallel over batch B=8 -> one batch element per core.

Per-core dataflow (all seq-major tensors transposed, d on partitions):
  xT [768,2048] --matmul(fp32r)--> qT|kT packed [128,2048] and vT [64,2048]
  scores wT[ks,q] = kT_blk.T @ qT  (row-group-alternated pairs: even ks
  block on PE rows 0-63, odd on 64-127 -> the two matmuls run on
  different sub-arrays concurrently and LDWEIGHTS overlaps)
  exp on ScalarE (scale=1/8 folded in; softmax max-subtraction skipped:
  score range is +-~2 so exp is exact), partial causal blocks multiplied
  by 0/1 mask patterns shipped from the host, fully masked blocks skipped.
  AV: outT[d,q] += v_blk[ks,d].T @ expT[ks,q], v augmented with a ones
  column so row 64 accumulates the softmax denominator for free; split
  into ks rows 0-63 / 64-127 (row groups again) accumulating into
  even/odd PSUM copies.
  Finalize per q-block: add even+odd, PE-transpose to natural [q, d+1],
  reciprocal of the denominator column, per-partition scalar multiply,
  one contiguous DMA of the natural-layout output.
"""

import sys

if "/opt/trn_rl_repo" not in sys.path:
    sys.path.insert(0, "/opt/trn_rl_repo")

import numpy as np

import concourse.bacc as bacc
import concourse.mybir as mybir
from concourse import tile
from concourse.bass_utils import run_bass_kernel_spmd

B, S, E, D = 8, 2048, 768, 64
EC = E // 128          # 6 e-chunks
NJ = 4                 # q blocks of 512
QW = S // NJ           # 512
NI = 16                # ks blocks of 128
KW = S // NI           # 128
SCALE = 1.0 / 8.0      # 1/sqrt(64)

F32 = mybir.dt.float32
F32R = mybir.dt.float32r
BF16 = mybir.dt.bfloat16

PROJ_F32R = True       # fp32r projections straight from f32 inputs


def _classify_mask(mask):
    """mask: [S,S] int (q,k indexed). Returns (blocks, patterns).

    blocks[J] = list of (i, pat_idx|None) ks-blocks included for q-block
    J. patterns: [128, QW] f32 arrays in wT layout (partition=ks, free=q)
    for partially masked blocks.
    """
    mb = (mask != 0).reshape(NJ, QW, NI, KW)
    sums = mb.sum(axis=(1, 3))
    patterns = []
    pat_ids = {}
    blocks = []
    for J in range(NJ):
        row = []
        for i in range(NI):
            s = int(sums[J, i])
            if s == 0:
                continue
            if s == QW * KW:
                row.append((i, None))
                continue
            pat = np.ascontiguousarray(mb[J, :, i, :].T.astype(np.float32))
            key = pat.tobytes()
            if key not in pat_ids:
                pat_ids[key] = len(patterns)
                patterns.append(pat)
            row.append((i, pat_ids[key]))
        if not row:
            raise ValueError(f"q-block {J} has no valid keys")
        blocks.append(row)
    return blocks, patterns


def _build(blocks, n_pat):
    nc = bacc.Bacc("TRN2", target_bir_lowering=False, debug=False, num_devices=B)

    xdt = F32R if PROJ_F32R else F32
    xT = nc.declare_dram_parameter("xT", [E, S], xdt, isOutput=False)
    wqkv = nc.declare_dram_parameter("wqkv", [E, 192], xdt, isOutput=False)
    ident = nc.declare_dram_parameter("ident", [128, 128], F32, isOutput=False)
    if n_pat:
        masks = nc.declare_dram_parameter(
            "masks", [128, n_pat * QW], F32, isOutput=False
        )
    out = nc.declare_dram_parameter("out", [S, D], F32, isOutput=True)

    xT_v = xT.ap().rearrange("(a p) s -> p a s", p=128)
    w_v = wqkv.ap().rearrange("(a p) d -> p a d", p=128)
    out_v = out.ap().rearrange("(t p) d -> p t d", p=128)  # [128, NI, D]

    EXP = mybir.ActivationFunctionType.Exp
    COPY = mybir.ActivationFunctionType.Copy
    ADD = mybir.AluOpType.add
    PSUM = "PSUM"

    with tile.TileContext(nc) as tc:
        with tc.tile_pool(name="perm", bufs=1) as perm, \
             tc.tile_pool(name="vpool", bufs=NI) as vpool, \
             tc.tile_pool(name="expp", bufs=3) as expp, \
             tc.tile_pool(name="smallp", bufs=2) as smallp:

            xt_sb = perm.tile([128, EC, S], xdt, tag="xt")
            w_sb = perm.tile([128, EC, 192], xdt, tag="w")
            ident_sb = perm.tile([128, 128], F32, tag="ident")
            qk_sb = perm.tile([128, S], BF16, tag="qk")      # qT @ rows 0:64, kT @ 64:128
            ktqh_sb = perm.tile([128, S], BF16, tag="ktqh")  # kT @ rows 0:64, qT @ 64:128
            vt_sb = perm.tile([64, S], BF16, tag="vt")
            if n_pat:
                mask_sb = perm.tile([128, n_pat, QW], BF16, tag="masks")
            of_sb = perm.tile([128, NI, D], F32, tag="of")   # natural-layout output

            # ---- loads ----
            nc.sync.dma_start(w_sb[:], w_v[:])
            nc.sync.dma_start(ident_sb[:], ident.ap()[:])
            for c in range(EC):
                nc.sync.dma_start(xt_sb[:, c, :], xT_v[:, c, :])
            if n_pat:
                nc.gpsimd.dma_start(  # SWDGE cast f32 -> bf16
                    mask_sb[:], masks.ap().rearrange("p (m s) -> p m s", s=QW)
                )

            # ---- projections ----
            with tc.tile_pool(name="projp", bufs=1, space=PSUM) as projp:
                qkp = projp.tile([128, S], F32, tag="qkp")
                vtp = projp.tile([64, S], F32, tag="vtp")
                for c in range(EC):
                    for h in range(4):
                        sl = slice(h * 512, (h + 1) * 512)
                        nc.tensor.matmul(
                            qkp[:, sl], w_sb[:, c, 0:128], xt_sb[:, c, sl],
                            start=(c == 0), stop=(c == EC - 1),
                        )
                    for h in range(4):
                        sl = slice(h * 512, (h + 1) * 512)
                        nc.tensor.matmul(
                            vtp[:, sl], w_sb[:, c, 128:192], xt_sb[:, c, sl],
                            start=(c == 0), stop=(c == EC - 1),
                        )
                nc.vector.tensor_copy(qk_sb[:], qkp[:])
                nc.scalar.activation(vt_sb[:], vtp[:], COPY)

            # swapped-row copy (cross-partition => SBUF-to-SBUF DMA)
            nc.sync.dma_start(ktqh_sb[0:64, :], qk_sb[64:128, :])
            nc.sync.dma_start(ktqh_sb[64:128, :], qk_sb[0:64, :])

            # v natural layout via xbar transpose + ones column
            v_tiles = []
            for t in range(NI):
                vt_t = vpool.tile([128, D + 1], BF16, tag="v")
                eng = nc.scalar if (t % 2 == 0) else nc.sync
                eng.dma_start(
                    vt_t[:, 0:D], vt_sb[:, t * KW:(t + 1) * KW], transpose=True
                )
                nc.vector.memset(vt_t[:, D:D + 1], 1.0)
                v_tiles.append(vt_t)

            # ---- attention ----
            with tc.tile_pool(name="wp", bufs=2, space=PSUM) as wp, \
                 tc.tile_pool(name="op", bufs=1, space=PSUM) as op, \
                 tc.tile_pool(name="tp", bufs=2, space=PSUM) as tp:
                for J in range(NJ):
                    qsl = slice(J * QW, (J + 1) * QW)
                    row = blocks[J]
                    pairs = [row[t:t + 2] for t in range(0, len(row), 2)]
                    o_e = op.tile([D + 1, QW], F32, tag="oe")
                    o_o = op.tile([D + 1, QW], F32, tag="oo")
                    n_lo = n_hi = 0
                    lo_tot = len(row)
                    hi_tot = len(row)
                    for pair in pairs:
                        w_ps = wp.tile([128, QW * len(pair)], F32, tag="w")
                        et = expp.tile([128, QW * len(pair)], BF16, tag="e")
                        for s_idx, (i, _) in enumerate(pair):
                            ksl = slice(i * KW, (i + 1) * KW)
                            osl = slice(s_idx * QW, (s_idx + 1) * QW)
                            if s_idx % 2 == 0:
                                nc.tensor.matmul(  # PE rows 0-63
                                    w_ps[:, osl], ktqh_sb[0:64, ksl],
                                    qk_sb[0:64, qsl], start=True, stop=True,
                                )
                            else:
                                nc.tensor.matmul(  # PE rows 64-127, concurrent
                                    w_ps[:, osl], qk_sb[64:128, ksl],
                                    ktqh_sb[64:128, qsl], start=True, stop=True,
                                )
                        nc.scalar.activation(et[:], w_ps[:], EXP, scale=SCALE)
                        for s_idx, (i, pat) in enumerate(pair):
                            if pat is not None:
                                esl = slice(s_idx * QW, (s_idx + 1) * QW)
                                nc.vector.tensor_mul(
                                    et[:, esl], et[:, esl], mask_sb[:, pat, :]
                                )
                        for s_idx, (i, _) in enumerate(pair):
                            esl = slice(s_idx * QW, (s_idx + 1) * QW)
                            n_lo += 1
                            nc.tensor.matmul(  # ks rows 0-63 -> even acc
                                o_e[:], v_tiles[i][0:64, 0:D + 1],
                                et[0:64, esl],
                                start=(n_lo == 1), stop=(n_lo == lo_tot),
                            )
                            n_hi += 1
                            nc.tensor.matmul(  # ks rows 64-127 -> odd acc
                                o_o[:], v_tiles[i][64:128, 0:D + 1],
                                et[64:128, esl],
                                start=(n_hi == 1), stop=(n_hi == hi_tot),
                            )
                    # ---- finalize q-block ----
                    ofb = smallp.tile([D + 1, QW], F32, tag="ofb")
                    oc = smallp.tile([D + 1, QW], F32, tag="oc")
                    nc.scalar.activation(oc[:], o_o[:], COPY)
                    nc.vector.tensor_tensor(ofb[:], o_e[:], oc[:], ADD)
                    for cblk in range(QW // 128):
                        tpt = tp.tile([128, D + 1], F32, tag="t")
                        nc.tensor.transpose(
                            tpt[:],
                            ofb[:, cblk * 128:(cblk + 1) * 128],
                            ident_sb[0:D + 1, 0:D + 1],
                        )
                        rcp = smallp.tile([128, 1], F32, tag="rcp")
                        nc.vector.reciprocal(rcp[:], tpt[:, D:D + 1])
                        nc.vector.tensor_scalar_mul(
                            of_sb[:, J * 4 + cblk, :], tpt[:, 0:D], rcp[:]
                        )
            nc.sync.dma_start(out_v[:], of_sb[:])

    nc.compile()
    return nc


_CACHE = {}


def kernel(inputs, attention_mask, Q, K, V):
    inputs = np.asarray(inputs, dtype=np.float32)
    Q = np.asarray(Q, dtype=np.float32)
    K = np.asarray(K, dtype=np.float32)
    V = np.asarray(V, dtype=np.float32)
    mask = np.asarray(attention_mask)
    assert inputs.shape == (B, S, E)
    assert mask.shape[-2:] == (S, S)

    blocks, patterns = _classify_mask(mask.reshape(S, S))
    n_pat = len(patterns)

    key = (tuple(tuple(r) for r in blocks), n_pat)
    if key not in _CACHE:
        _CACHE[key] = _build(blocks, n_pat)
    nc = _CACHE[key]

    wqkv = np.ascontiguousarray(np.concatenate([Q, K, V], axis=1))
    identity = np.eye(128, dtype=np.float32)
    if n_pat:
        mask_packed = np.ascontiguousarray(np.concatenate(patterns, axis=1))

    in_maps = []
    for b in range(B):
        m = {
            "xT": np.ascontiguousarray(inputs[b].T),
            "wqkv": wqkv,
            "ident": identity,
        }
        if n_pat:
            m["masks"] = mask_packed
        in_maps.append(m)

    res = run_bass_kernel_spmd(nc, in_maps, core_ids=list(range(B)))
    global _LAST_RESULTS
    _LAST_RESULTS = res
    out = np.stack([res.results[b]["out"] for b in range(B)], axis=0)
    return np.ascontiguousarray(out.astype(np.float32))


_LAST_RESULTS = None


if __name__ == "__main__":
    rng = np.random.default_rng(0)
    x = rng.standard_normal((B, S, E), dtype=np.float32)
    am = np.tril(np.ones((S, S), dtype=np.int32))[None]
    Q = rng.standard_normal((E, D), dtype=np.float32) * 0.01
    K = rng.standard_normal((E, D), dtype=np.float32) * 0.01
    V = rng.standard_normal((E, D), dtype=np.float32) * 0.01
    o = kernel(x, am, Q, K, V)
    print(o.shape, o.dtype)


# revision 7
# speedup vs baseline: 1.3665x; 1.2932x over previous
"""Distributed causal attention head for Trainium2 (8 NeuronCores).

Problem: inputs [8,2048,768] f32, attention_mask [1,2048,2048] int32,
Q/K/V [768,64] f32 -> out [8,2048,64] f32
  q,k,v = x@Q, x@K, x@V ; w = q k^T / 8 masked ; out = softmax(w) @ v

Sharding: data-parallel over batch B=8 -> one batch element per core.

Per-core dataflow (seq-major tensors transposed, d on partitions):
  xT [768,2048] --matmul(fp32r)--> qT|kT packed [128,2048] and vT [64,2048]
  scores wT[ks,q] = kT_blk.T @ qT with ks-block pairs alternated across
  PE row groups 0-63 / 64-127 (concurrent sub-arrays, LDW overlap);
  exp on ScalarE (scale=1/8 folded; max-subtraction skipped: scores are
  O(1) so exp is exact); partially-masked causal blocks: zero-prefix
  memset + narrow 0/1-mask multiply; fully-masked blocks skipped.
  AV: outT[d,q] += v_blk[ks,d].T @ expT[ks,q], v augmented with a ones
  column so row 64 accumulates the softmax denominator; ks rows split
  0-63 / 64-127 into even/odd PSUM accumulators (row groups again).
  Finalize per q-block: merge even+odd, PE-transpose to natural [q,d+1],
  reciprocal of denominator column, per-partition scalar multiply, one
  contiguous natural-layout output DMA.
"""

import sys

if "/opt/trn_rl_repo" not in sys.path:
    sys.path.insert(0, "/opt/trn_rl_repo")

import numpy as np

import concourse.bacc as bacc
import concourse.mybir as mybir
from concourse import tile
from concourse.bass_utils import run_bass_kernel_spmd

B, S, E, D = 8, 2048, 768, 64
EC = E // 128          # 6 e-chunks
NJ = 4                 # q blocks of 512
QW = S // NJ           # 512
NI = 16                # ks blocks of 128
KW = S // NI           # 128
SCALE = 1.0 / 8.0      # 1/sqrt(64)

F32 = mybir.dt.float32
F32R = mybir.dt.float32r
BF16 = mybir.dt.bfloat16


def _classify_mask(mask):
    """mask: [S,S] int (q,k indexed). Returns (blocks, patterns).

    blocks[J] = list of (i, pat_idx|None) ks-blocks included for q-block
    J.  patterns: list of (z, mid) where the block's mask in wT layout
    [128 ks, QW q] is [zeros(:, :z) | mid | ones]; mid is [128, mw] f32.
    """
    mb = (mask != 0).reshape(NJ, QW, NI, KW)
    sums = mb.sum(axis=(1, 3))
    patterns = []
    pat_ids = {}
    blocks = []
    for J in range(NJ):
        row = []
        for i in range(NI):
            s = int(sums[J, i])
            if s == 0:
                continue
            if s == QW * KW:
                row.append((i, None))
                continue
            pat = mb[J, :, i, :].T.astype(np.float32)  # [KW, QW]
            colfull = pat.all(axis=0)
            colzero = ~pat.any(axis=0)
            z = 0
            while z < QW and colzero[z]:
                z += 1
            e = QW
            while e > z and colfull[e - 1]:
                e -= 1
            mid = np.ascontiguousarray(pat[:, z:e])
            key = (z, mid.tobytes())
            if key not in pat_ids:
                pat_ids[key] = len(patterns)
                patterns.append((z, mid))
            row.append((i, pat_ids[key]))
        if not row:
            raise ValueError(f"q-block {J} has no valid keys")
        blocks.append(row)
    return blocks, patterns


def _build(blocks, patterns):
    n_pat = len(patterns)
    pat_off = []
    o = 0
    for z, mid in patterns:
        pat_off.append(o)
        o += mid.shape[1]
    masks_w = o

    nc = bacc.Bacc("TRN2", target_bir_lowering=False, debug=False, num_devices=B)

    xT = nc.declare_dram_parameter("xT", [E, S], F32R, isOutput=False)
    wqkv = nc.declare_dram_parameter("wqkv", [E, 192], F32R, isOutput=False)
    ident = nc.declare_dram_parameter("ident", [128, 128], F32, isOutput=False)
    if n_pat:
        masks = nc.declare_dram_parameter("masks", [128, masks_w], F32, isOutput=False)
    out = nc.declare_dram_parameter("out", [S, D], F32, isOutput=True)

    xT_v = xT.ap().rearrange("(a p) s -> p a s", p=128)
    w_v = wqkv.ap().rearrange("(a p) d -> p a d", p=128)
    out_v = out.ap().rearrange("(t p) d -> p t d", p=128)  # [128, NI, D]

    EXP = mybir.ActivationFunctionType.Exp
    COPY = mybir.ActivationFunctionType.Copy
    ADD = mybir.AluOpType.add
    PSUM = "PSUM"

    with tile.TileContext(nc) as tc:
        with tc.tile_pool(name="perm", bufs=1) as perm, \
             tc.tile_pool(name="qkp4", bufs=4) as qkp4, \
             tc.tile_pool(name="ktq4", bufs=4) as ktq4, \
             tc.tile_pool(name="vpool", bufs=NI) as vpool, \
             tc.tile_pool(name="expp", bufs=3) as expp, \
             tc.tile_pool(name="smallp", bufs=2) as smallp:

            xt_sb = perm.tile([128, EC, S], F32R, tag="xt")
            w_sb = perm.tile([128, EC, 192], F32R, tag="w")
            ident_sb = perm.tile([128, 128], F32, tag="ident")
            # quarter tiles: qkq[h] holds qT@rows0:64 | kT@rows64:128 for
            # q/k columns h*512..h*512+512; ktq[h] the row-swapped copy.
            qkq = [qkp4.tile([128, QW], BF16, tag="qk", name=f"qkq{h}") for h in range(4)]
            ktq = [ktq4.tile([128, QW], BF16, tag="ktq", name=f"ktq{h}") for h in range(4)]
            vt_sb = perm.tile([64, S], BF16, tag="vt")
            if n_pat:
                mask_sb = perm.tile([128, masks_w], BF16, tag="masks")
            of_sb = perm.tile([128, NI, D], F32, tag="of")

            # ---- loads ----
            nc.sync.dma_start(w_sb[:], w_v[:])
            nc.sync.dma_start(ident_sb[:], ident.ap()[:])
            for c in range(EC):
                nc.sync.dma_start(xt_sb[:, c, :], xT_v[:, c, :])
            if n_pat:
                nc.gpsimd.dma_start(mask_sb[:], masks.ap()[:])  # SWDGE f32->bf16

            # ---- projections (fp32r, full-rate at N=512) ----
            with tc.tile_pool(name="projp", bufs=1, space=PSUM) as projp:
                qkp = projp.tile([128, S], F32, tag="qkp")
                vtp = projp.tile([64, S], F32, tag="vtp")
                for c in range(EC):
                    for h in range(4):
                        sl = slice(h * 512, (h + 1) * 512)
                        nc.tensor.matmul(
                            qkp[:, sl], w_sb[:, c, 0:128], xt_sb[:, c, sl],
                            start=(c == 0), stop=(c == EC - 1),
                        )
                    for h in range(4):
                        sl = slice(h * 512, (h + 1) * 512)
                        nc.tensor.matmul(
                            vtp[:, sl], w_sb[:, c, 128:192], xt_sb[:, c, sl],
                            start=(c == 0), stop=(c == EC - 1),
                        )
                nc.scalar.activation(vt_sb[:], vtp[:], COPY)
                for h in range(4):
                    nc.vector.tensor_copy(qkq[h][:], qkp[:, h * QW:(h + 1) * QW])

            # row-swapped quarters (cross-partition => SBUF-to-SBUF DMA) and
            # v tiles (xbar transpose + ones column), interleaved on sync.
            v_tiles = [vpool.tile([128, D + 1], BF16, tag="v", name=f"v{t}") for t in range(NI)]
            for t in range(NI):
                nc.vector.memset(v_tiles[t][:, D:D + 1], 1.0)
            for h in range(4):
                nc.sync.dma_start(ktq[h][0:64, :], qkq[h][64:128, :])
                nc.sync.dma_start(ktq[h][64:128, :], qkq[h][0:64, :])
                for t in range(4 * h, 4 * h + 4):
                    nc.sync.dma_start(
                        v_tiles[t][:, 0:D],
                        vt_sb[:, t * KW:(t + 1) * KW],
                        transpose=True,
                    )

            # ---- attention ----
            with tc.tile_pool(name="wp", bufs=2, space=PSUM) as wp, \
                 tc.tile_pool(name="op", bufs=1, space=PSUM) as op, \
                 tc.tile_pool(name="tp", bufs=2, space=PSUM) as tp:
                for J in range(NJ):
                    row = blocks[J]
                    pairs = [row[t:t + 2] for t in range(0, len(row), 2)]
                    o_e = op.tile([D + 1, QW], F32, tag="oe")
                    o_o = op.tile([D + 1, QW], F32, tag="oo")
                    n_lo = n_hi = 0
                    tot = len(row)
                    for pair in pairs:
                        w_ps = wp.tile([128, QW * len(pair)], F32, tag="w")
                        et = expp.tile([128, QW * len(pair)], BF16, tag="e")
                        for s_idx, (i, _) in enumerate(pair):
                            kq, kr = divmod(i, 4)
                            ksl = slice(kr * KW, (kr + 1) * KW)
                            osl = slice(s_idx * QW, (s_idx + 1) * QW)
                            if s_idx % 2 == 0:
                                nc.tensor.matmul(  # PE rows 0-63
                                    w_ps[:, osl], ktq[kq][0:64, ksl],
                                    qkq[J][0:64, :], start=True, stop=True,
                                )
                            else:
                                nc.tensor.matmul(  # PE rows 64-127, concurrent
                                    w_ps[:, osl], qkq[kq][64:128, ksl],
                                    ktq[J][64:128, :], start=True, stop=True,
                                )
                        nc.scalar.activation(et[:], w_ps[:], EXP, scale=SCALE)
                        for s_idx, (i, pat) in enumerate(pair):
                            if pat is not None:
                                z, mid = patterns[pat]
                                mw = mid.shape[1]
                                base = s_idx * QW
                                if z:
                                    nc.vector.memset(et[:, base:base + z], 0.0)
                                nc.vector.tensor_mul(
                                    et[:, base + z:base + z + mw],
                                    et[:, base + z:base + z + mw],
                                    mask_sb[:, pat_off[pat]:pat_off[pat] + mw],
                                )
                        for s_idx, (i, _) in enumerate(pair):
                            esl = slice(s_idx * QW, (s_idx + 1) * QW)
                            n_lo += 1
                            nc.tensor.matmul(  # ks rows 0-63 -> even acc
                                o_e[:], v_tiles[i][0:64, 0:D + 1],
                                et[0:64, esl],
                                start=(n_lo == 1), stop=(n_lo == tot),
                            )
                            n_hi += 1
                            nc.tensor.matmul(  # ks rows 64-127 -> odd acc
                                o_o[:], v_tiles[i][64:128, 0:D + 1],
                                et[64:128, esl],
                                start=(n_hi == 1), stop=(n_hi == tot),
                            )
                    # ---- finalize q-block ----
                    ofb = smallp.tile([D + 1, QW], F32, tag="ofb")
                    oc = smallp.tile([D + 1, QW], F32, tag="oc")
                    nc.vector.tensor_copy(oc[:], o_o[:])
                    nc.vector.tensor_tensor(ofb[:], o_e[:], oc[:], ADD)
                    for cblk in range(QW // 128):
                        tpt = tp.tile([128, D + 1], F32, tag="t")
                        nc.tensor.transpose(
                            tpt[:],
                            ofb[:, cblk * 128:(cblk + 1) * 128],
                            ident_sb[0:D + 1, 0:D + 1],
                        )
                        rcp = smallp.tile([128, 1], F32, tag="rcp")
                        nc.vector.reciprocal(rcp[:], tpt[:, D:D + 1])
                        nc.vector.tensor_scalar_mul(
                            of_sb[:, J * 4 + cblk, :], tpt[:, 0:D], rcp[:]
                        )
            nc.sync.dma_start(out_v[:], of_sb[:])

    nc.compile()
    return nc


_CACHE = {}


def kernel(inputs, attention_mask, Q, K, V):
    inputs = np.asarray(inputs, dtype=np.float32)
    Q = np.asarray(Q, dtype=np.float32)
    K = np.asarray(K, dtype=np.float32)
    V = np.asarray(V, dtype=np.float32)
    mask = np.asarray(attention_mask)
    assert inputs.shape == (B, S, E)
    assert mask.shape[-2:] == (S, S)

    blocks, patterns = _classify_mask(mask.reshape(S, S))

    key = (
        tuple(tuple(r) for r in blocks),
        tuple((z, m.tobytes()) for z, m in patterns),
    )
    if key not in _CACHE:
        _CACHE[key] = _build(blocks, patterns)
    nc = _CACHE[key]

    wqkv = np.ascontiguousarray(np.concatenate([Q, K, V], axis=1))
    identity = np.eye(128, dtype=np.float32)
    if patterns:
        mask_packed = np.ascontiguousarray(
            np.concatenate([m for _, m in patterns], axis=1)
        )

    in_maps = []
    for b in range(B):
        m = {
            "xT": np.ascontiguousarray(inputs[b].T),
            "wqkv": wqkv,
            "ident": identity,
        }
        if patterns:
            m["masks"] = mask_packed
        in_maps.append(m)

    res = run_bass_kernel_spmd(nc, in_maps, core_ids=list(range(B)))
    global _LAST_RESULTS
    _LAST_RESULTS = res
    out = np.stack([res.results[b]["out"] for b in range(B)], axis=0)
    return np.ascontiguousarray(out.astype(np.float32))


_LAST_RESULTS = None


if __name__ == "__main__":
    rng = np.random.default_rng(0)
    x = rng.standard_normal((B, S, E), dtype=np.float32)
    am = np.tril(np.ones((S, S), dtype=np.int32))[None]
    Q = rng.standard_normal((E, D), dtype=np.float32) * 0.01
    K = rng.standard_normal((E, D), dtype=np.float32) * 0.01
    V = rng.standard_normal((E, D), dtype=np.float32) * 0.01
    o = kernel(x, am, Q, K, V)
    print(o.shape, o.dtype)


# revision 9
# speedup vs baseline: 1.4049x; 1.0281x over previous
"""Distributed causal attention head for Trainium2 (8 NeuronCores).

Problem: inputs [8,2048,768] f32, attention_mask [1,2048,2048] int32,
Q/K/V [768,64] f32 -> out [8,2048,64] f32
  q,k,v = x@Q, x@K, x@V ; w = q k^T / 8 masked ; out = softmax(w) @ v

Sharding: data-parallel over batch B=8 -> one batch element per core.

Per-core dataflow (seq-major tensors transposed, d on partitions):
  xT [768,2048] --matmul(fp32r)--> qT|kT packed quarters and vT [64,2048]
  scores wT[ks,q] = kT_blk.T @ qT with ks-block pairs alternated across
  PE row groups 0-63 / 64-127 (adjacent matmuls overlap on different
  sub-arrays); exp on ScalarE (scale=1/8 folded; max-subtraction skipped:
  scores are O(1) so exp is exact); partially-masked causal blocks get a
  zero-prefix memset + narrow 0/1 mask multiply; fully-masked blocks are
  skipped. v reaches natural [ks, d] layout via PE transposes of vT.
  AV: outT[d,q] += v_blk[ks,d].T @ expT[ks,q] with the ks contraction
  split rows 0-63 / 64-127 into even/odd PSUM accumulators (row-group
  concurrency again); a ones column on v accumulates the softmax
  denominator in row 64.  Finalize per 128 queries: merge even+odd,
  PE-transpose to natural [q, d+1], reciprocal of the denominator
  column, per-partition scalar multiply, one natural-layout output DMA.
"""

import sys

if "/opt/trn_rl_repo" not in sys.path:
    sys.path.insert(0, "/opt/trn_rl_repo")

import numpy as np

import concourse.bacc as bacc
import concourse.mybir as mybir
from concourse import tile
from concourse.bass_utils import run_bass_kernel_spmd

B, S, E, D = 8, 2048, 768, 64
EC = E // 128          # 6 e-chunks
NJ = 4                 # q blocks of 512
QW = S // NJ           # 512
NI = 16                # ks blocks of 128
KW = S // NI           # 128
SCALE = 1.0 / 8.0      # 1/sqrt(64)

F32 = mybir.dt.float32
F32R = mybir.dt.float32r
BF16 = mybir.dt.bfloat16


def _classify_mask(mask):
    """mask: [S,S] int (q,k indexed). Returns (blocks, patterns).

    blocks[J] = list of (i, pat_idx|None) ks-blocks included for q-block
    J.  patterns: list of (z, mid): the block's mask in wT layout
    [128 ks, QW q] is [zeros(:, :z) | mid | ones]; mid is [128, mw] f32.
    """
    mb = (mask != 0).reshape(NJ, QW, NI, KW)
    sums = mb.sum(axis=(1, 3))
    patterns = []
    pat_ids = {}
    blocks = []
    for J in range(NJ):
        row = []
        for i in range(NI):
            s = int(sums[J, i])
            if s == 0:
                continue
            if s == QW * KW:
                row.append((i, None))
                continue
            pat = mb[J, :, i, :].T.astype(np.float32)  # [KW, QW]
            colfull = pat.all(axis=0)
            colzero = ~pat.any(axis=0)
            z = 0
            while z < QW and colzero[z]:
                z += 1
            e = QW
            while e > z and colfull[e - 1]:
                e -= 1
            mid = np.ascontiguousarray(pat[:, z:e])
            key = (z, mid.tobytes())
            if key not in pat_ids:
                pat_ids[key] = len(patterns)
                patterns.append((z, mid))
            row.append((i, pat_ids[key]))
        if not row:
            raise ValueError(f"q-block {J} has no valid keys")
        blocks.append(row)
    return blocks, patterns


def _build(blocks, patterns):
    n_pat = len(patterns)
    pat_off = []
    o = 0
    for z, mid in patterns:
        pat_off.append(o)
        o += mid.shape[1]
    masks_w = o

    nc = bacc.Bacc("TRN2", target_bir_lowering=False, debug=False, num_devices=B)

    xT = nc.declare_dram_parameter("xT", [E, S], F32R, isOutput=False)
    wqkv = nc.declare_dram_parameter("wqkv", [E, 192], F32R, isOutput=False)
    ident = nc.declare_dram_parameter("ident", [128, 128], F32, isOutput=False)
    if n_pat:
        masks = nc.declare_dram_parameter("masks", [128, masks_w], F32, isOutput=False)
    out = nc.declare_dram_parameter("out", [S, D], F32, isOutput=True)

    xT_v = xT.ap().rearrange("(a p) s -> p a s", p=128)
    w_v = wqkv.ap().rearrange("(a p) d -> p a d", p=128)
    out_v = out.ap().rearrange("(t p) d -> p t d", p=128)  # [128, NI, D]

    EXP = mybir.ActivationFunctionType.Exp
    ADD = mybir.AluOpType.add
    PSUM = "PSUM"

    with tile.TileContext(nc) as tc:
        with tc.tile_pool(name="perm", bufs=1) as perm, \
             tc.tile_pool(name="qkp4", bufs=4) as qkp4, \
             tc.tile_pool(name="ktq4", bufs=4) as ktq4, \
             tc.tile_pool(name="vpool", bufs=NI) as vpool, \
             tc.tile_pool(name="expp", bufs=3) as expp, \
             tc.tile_pool(name="smallp", bufs=2) as smallp:

            xt_sb = perm.tile([128, EC, S], F32R, tag="xt")
            w_sb = perm.tile([128, EC, 192], F32R, tag="w")
            ident_sb = perm.tile([128, 128], F32, tag="ident")
            ident_bf = perm.tile([128, 128], BF16, tag="identbf")
            # qkq[h]: qT@rows0:64 | kT@rows64:128, q/k cols h*512..+512;
            # ktq[h]: the row-swapped copy (kT@lo | qT@hi).
            qkq = [qkp4.tile([128, QW], BF16, tag="qk", name=f"qkq{h}")
                   for h in range(4)]
            ktq = [ktq4.tile([128, QW], BF16, tag="ktq", name=f"ktq{h}")
                   for h in range(4)]
            vt_sb = perm.tile([64, S], BF16, tag="vt")
            if n_pat:
                mask_sb = perm.tile([128, masks_w], BF16, tag="masks")
            of_sb = perm.tile([128, NI, D], F32, tag="of")

            # ---- loads ----
            nc.sync.dma_start(w_sb[:], w_v[:])
            for c in range(EC):
                nc.sync.dma_start(xt_sb[:, c, :], xT_v[:, c, :])
            nc.sync.dma_start(ident_sb[:], ident.ap()[:])
            if n_pat:
                nc.gpsimd.dma_start(mask_sb[:], masks.ap()[:])  # SWDGE f32->bf16
            nc.vector.tensor_copy(ident_bf[:], ident_sb[:])

            # ---- projections (fp32r runs full-rate at N=512) ----
            with tc.tile_pool(name="projp", bufs=1, space=PSUM) as projp:
                qkp = projp.tile([128, S], F32, tag="qkp")
                vtp = projp.tile([64, S], F32, tag="vtp")
                for c in range(EC):
                    for h in range(4):
                        sl = slice(h * 512, (h + 1) * 512)
                        nc.tensor.matmul(
                            qkp[:, sl], w_sb[:, c, 0:128], xt_sb[:, c, sl],
                            start=(c == 0), stop=(c == EC - 1),
                        )
                    for h in range(4):
                        sl = slice(h * 512, (h + 1) * 512)
                        nc.tensor.matmul(
                            vtp[:, sl], w_sb[:, c, 128:192], xt_sb[:, c, sl],
                            start=(c == 0), stop=(c == EC - 1),
                        )
                vtq = []
                for h in range(4):
                    sl = slice(h * QW, (h + 1) * QW)
                    nc.vector.tensor_copy(qkq[h][:], qkp[:, sl])
                    nc.sync.dma_start(ktq[h][0:64, :], qkq[h][64:128, :])
                    nc.sync.dma_start(ktq[h][64:128, :], qkq[h][0:64, :])
                    vtq.append(h)
                    nc.vector.tensor_copy(vt_sb[:, sl], vtp[:, sl])

            v_tiles = [vpool.tile([128, D + 1], BF16, tag="v", name=f"v{t}")
                       for t in range(NI)]
            for t in range(NI):
                nc.vector.memset(v_tiles[t][:, D:D + 1], 1.0)

            # ---- attention ----
            with tc.tile_pool(name="vtr", bufs=1, space=PSUM) as vtr, \
                 tc.tile_pool(name="wp", bufs=2, space=PSUM) as wp, \
                 tc.tile_pool(name="op", bufs=1, space=PSUM) as op, \
                 tc.tile_pool(name="tp", bufs=1, space=PSUM) as tp:
                for J in range(NJ):
                    # v tiles 4J..4J+3 via PE transpose of vT (natural layout)
                    for t in range(4 * J, 4 * J + 4):
                        vtt = vtr.tile([128, KW // 2], BF16, tag="vtr",
                                       name=f"vtr{t}")
                        nc.tensor.transpose(
                            vtt[:, 0:D],
                            vt_sb[:, t * KW:(t + 1) * KW],
                            ident_bf[0:D, 0:D],
                        )
                        nc.vector.tensor_copy(v_tiles[t][:, 0:D], vtt[:, 0:D])

                    row = blocks[J]
                    pairs = [row[t:t + 2] for t in range(0, len(row), 2)]
                    o_e = op.tile([D + 1, QW], F32, tag="oe")
                    o_o = op.tile([D + 1, QW], F32, tag="oo")
                    n_lo = n_hi = 0
                    tot = len(row)
                    for pair in pairs:
                        w_ps = wp.tile([128, QW * len(pair)], F32, tag="w")
                        et = expp.tile([128, QW * len(pair)], BF16, tag="e")
                        for s_idx, (i, _) in enumerate(pair):
                            kq, kr = divmod(i, 4)
                            ksl = slice(kr * KW, (kr + 1) * KW)
                            osl = slice(s_idx * QW, (s_idx + 1) * QW)
                            if s_idx % 2 == 0:
                                nc.tensor.matmul(  # PE rows 0-63
                                    w_ps[:, osl], ktq[kq][0:64, ksl],
                                    qkq[J][0:64, :], start=True, stop=True,
                                )
                            else:
                                nc.tensor.matmul(  # PE rows 64-127
                                    w_ps[:, osl], qkq[kq][64:128, ksl],
                                    ktq[J][64:128, :], start=True, stop=True,
                                )
                        nc.scalar.activation(et[:], w_ps[:], EXP, scale=SCALE)
                        for s_idx, (i, pat) in enumerate(pair):
                            if pat is not None:
                                z, mid = patterns[pat]
                                mw = mid.shape[1]
                                base = s_idx * QW
                                if z:
                                    nc.vector.memset(et[:, base:base + z], 0.0)
                                nc.vector.tensor_mul(
                                    et[:, base + z:base + z + mw],
                                    et[:, base + z:base + z + mw],
                                    mask_sb[:, pat_off[pat]:pat_off[pat] + mw],
                                )
                        for s_idx, (i, _) in enumerate(pair):
                            esl = slice(s_idx * QW, (s_idx + 1) * QW)
                            n_lo += 1
                            nc.tensor.matmul(  # ks rows 0-63 -> even acc
                                o_e[:], v_tiles[i][0:64, 0:D + 1],
                                et[0:64, esl],
                                start=(n_lo == 1), stop=(n_lo == tot),
                            )
                            n_hi += 1
                            nc.tensor.matmul(  # ks rows 64-127 -> odd acc
                                o_o[:], v_tiles[i][64:128, 0:D + 1],
                                et[64:128, esl],
                                start=(n_hi == 1), stop=(n_hi == tot),
                            )
                    # ---- finalize q-block ----
                    ofb = smallp.tile([D + 1, QW], BF16, tag="ofb")
                    oc = smallp.tile([D + 1, QW], F32, tag="oc")
                    nc.vector.tensor_copy(oc[:], o_o[:])
                    nc.vector.tensor_tensor(ofb[:], o_e[:], oc[:], ADD)
                    for cblk in range(QW // 128):
                        tpt = tp.tile([128, D + 1], BF16, tag="t")
                        nc.tensor.transpose(
                            tpt[:],
                            ofb[:, cblk * 128:(cblk + 1) * 128],
                            ident_bf[0:D + 1, 0:D + 1],
                        )
                        rcp = smallp.tile([128, 1], F32, tag="rcp")
                        nc.vector.reciprocal(rcp[:], tpt[:, D:D + 1])
                        nc.vector.tensor_scalar_mul(
                            of_sb[:, J * 4 + cblk, :], tpt[:, 0:D], rcp[:]
                        )
            nc.sync.dma_start(out_v[:], of_sb[:])

    nc.compile()
    return nc


_CACHE = {}


def kernel(inputs, attention_mask, Q, K, V):
    inputs = np.asarray(inputs, dtype=np.float32)
    Q = np.asarray(Q, dtype=np.float32)
    K = np.asarray(K, dtype=np.float32)
    V = np.asarray(V, dtype=np.float32)
    mask = np.asarray(attention_mask)
    assert inputs.shape == (B, S, E)
    assert mask.shape[-2:] == (S, S)

    blocks, patterns = _classify_mask(mask.reshape(S, S))

    key = (
        tuple(tuple(r) for r in blocks),
        tuple((z, m.tobytes()) for z, m in patterns),
    )
    if key not in _CACHE:
        _CACHE[key] = _build(blocks, patterns)
    nc = _CACHE[key]

    wqkv = np.ascontiguousarray(np.concatenate([Q, K, V], axis=1))
    identity = np.eye(128, dtype=np.float32)
    if patterns:
        mask_packed = np.ascontiguousarray(
            np.concatenate([m for _, m in patterns], axis=1)
        )

    in_maps = []
    for b in range(B):
        m = {
            "xT": np.ascontiguousarray(inputs[b].T),
            "wqkv": wqkv,
            "ident": identity,
        }
        if patterns:
            m["masks"] = mask_packed
        in_maps.append(m)

    res = run_bass_kernel_spmd(nc, in_maps, core_ids=list(range(B)))
    global _LAST_RESULTS
    _LAST_RESULTS = res
    out = np.stack([res.results[b]["out"] for b in range(B)], axis=0)
    return np.ascontiguousarray(out.astype(np.float32))


_LAST_RESULTS = None


if __name__ == "__main__":
    rng = np.random.default_rng(0)
    x = rng.standard_normal((B, S, E), dtype=np.float32)
    am = np.tril(np.ones((S, S), dtype=np.int32))[None]
    Q = rng.standard_normal((E, D), dtype=np.float32) * 0.01
    K = rng.standard_normal((E, D), dtype=np.float32) * 0.01
    V = rng.standard_normal((E, D), dtype=np.float32) * 0.01
    o = kernel(x, am, Q, K, V)
    print(o.shape, o.dtype)
